# revision 40
# baseline (speedup 1.0000x reference)
"""MinusAttention kernel for Trainium2 (8 NeuronCores, Bass/Tile).

Math: score[i,j] = (w.q_i - w.k_j + b) / sqrt(E) with causal mask.
Within a softmax row i the w.q_i and b terms cancel, so

    weights[i,j] = g_j / sum_{j'<=i} g_j',   g_j = exp(-w.k_j / sqrt(E))
    out[i,:]     = (sum_{j<=i} g_j V[j,:]) / (sum_{j<=i} g_j)

i.e. a causal cumulative weighted average of V -- O(S*E) per (b,h) --
and the output does not depend on queries at all.

fp16 end-to-end (PE 1 cycle/col vs 4 for fp32, half the HBM bytes)
with k-major layouts everywhere so every engine/matmul access pattern
is contiguous (measured: transposed ACT writes cost 3.5x, permuted
matmul rhs costs 2x on this HW).  Ragged [7,7,2]-block PSUM chunks (3
per pair, each within a 2KB bank), one merged block-sum scatter per
pair, carry mask multiply in the DVE packed-16bit 2x mode, two-stage
reduce (packed TT + reduce), consts DMA'd from host.  kt+scatter ride
the SP HW ring, vg the ACT HW ring, output stores the gpsimd queue;
the first pair's inputs lead both queues since the first reduce gates
the pipeline.  Final normalize on GPSIMD for the first two pairs (DVE
is busy with front phases then), on DVE for the last two (tail).

Per pair (b,h), s = 128*k + p (p partition, k block 0..15):
  kt  [128,16,64] f16  (host-prescaled by -w/sqrt(E))
  vg  [128,16,65] f16  (col e=64 is ones)
  sk  = reduce_add_e(kt)            DVE 2-stage -> [128,16] f16
  g   = exp(sk)                     ACT
  wg  = vg * g_bcast                DVE
  ps_c = triT @ wg[:,k0:k1,:]       PE f16 (within-block prefix sums)
  c32[:,k0:k1,:] = ps_c[96:128]     ACT (PSUM reads: 32-aligned base)
  bsT = scatter c32 row 31          1 DMA (block sums -> partitions)
  rm  = maskKED * bsT_bcast         DVE 2x (carry terms, k' < k)
  ps_c += ones16 @ rm[:,k0:k1,:]    PE (adds inter-block carries)
  cw[:,k0:k1,:] = ps_c              ACT drains
  r   = 1/cw[:,:,64]                DVE
  ot  = cw[:,:,0:64] * r_bcast      GPSIMD / DVE
  out DMA (f16; host upcasts)
"""

import numpy as np

B, L, S, H, E = 4, 2048, 2048, 8, 64
NCORES = 8
PAIRS = (B * H) // NCORES  # 4 (b,h) pairs per core
NBLK = S // 128  # 16
# ragged PSUM chunks: [7,7,2] blocks -> 3 tiles/pair, each within a 2KB bank
CHUNKS = [(0, 7), (7, 14), (14, 16)]
GROUP = 2
SCALE = np.float32(1.0 / np.sqrt(np.float32(E)))

TRACE = False
LAST_RESULTS = None

_compiled = None


def _build():
    from concourse import bacc
    import concourse.mybir as mybir
    import concourse.tile as tile

    f16 = mybir.dt.float16
    f32 = mybir.dt.float32
    nc = bacc.Bacc("TRN2", target_bir_lowering=False, debug=False)

    ktw = nc.dram_tensor("ktw", [PAIRS, 128, NBLK, E], f16, kind="ExternalInput")
    vg = nc.dram_tensor("vg", [PAIRS, 128, NBLK, E + 1], f16, kind="ExternalInput")
    tri_c = nc.dram_tensor("tri_c", [128, 128], f16, kind="ExternalInput")
    ones_c = nc.dram_tensor("ones_c", [16, 128], f16, kind="ExternalInput")
    # maskKED[k', k, e] = 1 iff k' < k (bcast along e incl. the D col)
    mke_c = nc.dram_tensor("mke_c", [16, NBLK, E + 1], f16, kind="ExternalInput")
    out = nc.dram_tensor("out", [PAIRS, 128, NBLK, E], f16, kind="ExternalOutput")

    with tile.TileContext(nc) as tc:
        with (
            nc.allow_low_precision(reason="fp16 kernel; harness gate is 2e-2"),
            tc.tile_pool(name="const", bufs=1) as cpool,
            tc.tile_pool(name="ktp", bufs=PAIRS) as ktp,
            tc.tile_pool(name="vgp", bufs=PAIRS) as vgp,
            tc.tile_pool(name="skp", bufs=2 * GROUP) as skp,
            tc.tile_pool(name="gp", bufs=2 * GROUP) as gp,
            tc.tile_pool(name="wgp", bufs=PAIRS) as wgp,
            tc.tile_pool(name="c32p", bufs=2 * GROUP) as c32p,
            tc.tile_pool(name="bsp", bufs=2 * GROUP) as bsp,
            tc.tile_pool(name="rmp", bufs=2 * GROUP) as rmp,
            tc.tile_pool(name="rp", bufs=2 * GROUP) as rp,
            tc.tile_pool(name="cwp", bufs=GROUP + 1) as cwp,
            tc.tile_pool(name="otp", bufs=GROUP + 1) as otp,
            tc.tile_pool(name="ps", bufs=6, space="PSUM") as psp,
        ):
            allp = list(range(PAIRS))
            kts, vgts = {}, {}
            # first pair's inputs lead both queues: the first reduce gates
            # the whole pipeline, so nothing may be enqueued ahead of kt0
            for p in allp:
                kt = ktp.tile([128, NBLK, E], f16, tag="kt")
                vgt = vgp.tile([128, NBLK, E + 1], f16, tag="vg")
                nc.sync.dma_start(out=kt[:], in_=ktw[p])
                nc.scalar.dma_start(out=vgt[:], in_=vg[p])
                kts[p], vgts[p] = kt, vgt

            tri = cpool.tile([128, 128], f16)
            nc.sync.dma_start(out=tri[:], in_=tri_c[:])
            ones16 = cpool.tile([16, 128], f16)
            nc.sync.dma_start(out=ones16[:], in_=ones_c[:])
            maskKED = cpool.tile([16, NBLK, E + 1], f16)
            nc.sync.dma_start(out=maskKED[:], in_=mke_c[:])

            # front phases, paced pair-by-pair on DVE/ACT; the order edge
            # keeps the scheduler from running later pairs' reduces (gated
            # on late kt DMAs) ahead of this pair's wg in the DVE stream
            from concourse.tile_rust import add_dep_helper
            wgs = {}
            prev_wg = None
            for p in allp:
                # two-stage reduce: DVE adds the e-halves in the packed-16bit
                # 2x mode, the otherwise-idle GPSIMD engine finishes the sum
                t1 = skp.tile([128, NBLK, E // 2], f16, tag="t1")
                red = nc.vector.tensor_tensor(
                    out=t1[:], in0=kts[p][:, :, 0:E // 2],
                    in1=kts[p][:, :, E // 2:E], op=mybir.AluOpType.add,
                )
                sk = skp.tile([128, NBLK], f16, tag="sk")
                nc.vector.tensor_reduce(
                    sk[:], t1[:], mybir.AxisListType.X, mybir.AluOpType.add
                )
                if prev_wg is not None:
                    add_dep_helper(red.ins, prev_wg.ins, sync=False,
                                   reason="reduce after prev pair wg")
                g = gp.tile([128, NBLK], f16, tag="g")
                nc.scalar.activation(
                    g[:], sk[:], mybir.ActivationFunctionType.Exp
                )
                wg = wgp.tile([128, NBLK, E + 1], f16, tag="wg")
                prev_wg = nc.vector.tensor_tensor(
                    out=wg[:], in0=vgts[p][:],
                    in1=g[:].to_broadcast([128, NBLK, E + 1]),
                    op=mybir.AluOpType.mult,
                )
                wgs[p] = wg

            # PSUM stages: two pairs in flight (3 banks each)
            for grp in range(PAIRS // GROUP):
                pairs = list(range(grp * GROUP, (grp + 1) * GROUP))

                pss = {}
                for p in pairs:
                    for ci, (k0, k1) in enumerate(CHUNKS):
                        ps = psp.tile([128, k1 - k0, E + 1], f32, tag="ps")
                        nc.tensor.matmul(
                            ps[:], lhsT=tri[:],
                            rhs=wgs[p][:, k0:k1, :],
                            start=True, stop=False, skip_group_check=True,
                        )
                        pss[(p, ci)] = ps

                bsTs = {}
                for p in pairs:
                    # block sums live in row 127; PSUM reads need 32-aligned
                    # partition base: copy rows 96:128, then scatter row 31
                    c32 = c32p.tile([32, NBLK, E + 1], f16, tag="c32")
                    for ci, (k0, k1) in enumerate(CHUNKS):
                        nc.scalar.copy(
                            c32[:, k0:k1, :], pss[(p, ci)][96:128, :, :])
                    bsT = bsp.tile([NBLK, 1, E + 1], f16, tag="bs")
                    nc.sync.dma_start(out=bsT[:], in_=c32[31:32, :, :],
                                      single_packet=True)
                    bsTs[p] = bsT

                rms = {}
                for p in pairs:
                    rm = rmp.tile([NBLK, NBLK, E + 1], f16, tag="rm")
                    nc.vector.tensor_tensor(
                        out=rm[:], in0=maskKED[:],
                        in1=bsTs[p][:].broadcast_to([NBLK, NBLK, E + 1]),
                        op=mybir.AluOpType.mult,
                    )
                    rms[p] = rm

                for p in pairs:
                    for ci, (k0, k1) in enumerate(CHUNKS):
                        nc.tensor.matmul(
                            pss[(p, ci)][:], lhsT=ones16[:],
                            rhs=rms[p][:, k0:k1, :],
                            start=False, stop=True, skip_group_check=True,
                        )

                for p in pairs:
                    cw = cwp.tile([128, NBLK, E + 1], f16, tag="cw")
                    for ci, (k0, k1) in enumerate(CHUNKS):
                        if ci == len(CHUNKS) - 1:
                            nc.vector.tensor_copy(
                                cw[:, k0:k1, :], pss[(p, ci)][:])
                        else:
                            nc.scalar.copy(cw[:, k0:k1, :], pss[(p, ci)][:])
                    r = rp.tile([128, NBLK], f16, tag="r")
                    nc.vector.reciprocal(
                        r[:], cw[:, :, E:E + 1].rearrange("p k o -> p (k o)"))
                    ot = otp.tile([128, NBLK, E], f16, tag="ot")
                    # final normalize: Pool for the first stage (DVE is busy
                    # with the front phases then), DVE for the tail stage
                    eng = nc.gpsimd if grp == 0 else nc.vector
                    eng.tensor_tensor(
                        out=ot[:], in0=cw[:, :, 0:E],
                        in1=r[:].to_broadcast([128, NBLK, E]),
                        op=mybir.AluOpType.mult,
                    )
                    nc.gpsimd.dma_start(out=out[p], in_=ot[:])

    nc.compile()
    return nc


def _get_compiled():
    global _compiled
    if _compiled is None:
        _compiled = _build()
    return _compiled


def _consts():
    f16 = np.float16
    tri = np.triu(np.ones((128, 128), np.float32)).astype(f16)  # tri[c,p]=1 iff c<=p
    ones16 = np.ones((16, 128), f16)
    mk = (np.arange(NBLK)[:, None] < np.arange(NBLK)[None, :]).astype(np.float32)
    mke = np.broadcast_to(mk[:, :, None], (16, NBLK, E + 1)).astype(f16)
    return {
        "tri_c": tri,
        "ones_c": ones16,
        "mke_c": np.ascontiguousarray(mke),
    }


def prep_inputs(keys: np.ndarray, values: np.ndarray, w_score: np.ndarray):
    """Host-side reshard: returns in_maps (list of 8 dicts)."""
    keys = np.asarray(keys, dtype=np.float32)
    values = np.asarray(values, dtype=np.float32)
    w = np.asarray(w_score, dtype=np.float32)

    # [B,S,H,E] -> [B,H,S,E] -> [B*H, NBLK, 128, E] -> [B*H, 128, NBLK, E]
    kt = keys.transpose(0, 2, 1, 3).reshape(B * H, NBLK, 128, E)
    kt = (kt * (-SCALE * w)).transpose(0, 2, 1, 3).astype(np.float16)

    v = values.transpose(0, 2, 1, 3).reshape(B * H, NBLK, 128, E)
    v = v.transpose(0, 2, 1, 3)  # [B*H, 128, NBLK, E]
    vgf = np.concatenate(
        [v, np.ones((B * H, 128, NBLK, 1), np.float32)], axis=3
    ).astype(np.float16)  # [B*H, 128, NBLK, E+1]

    consts = _consts()
    in_maps = []
    for c in range(NCORES):
        sl = slice(PAIRS * c, PAIRS * (c + 1))
        m = {
            "ktw": np.ascontiguousarray(kt[sl]),
            "vg": np.ascontiguousarray(vgf[sl]),
        }
        m.update(consts)
        in_maps.append(m)
    return in_maps


def assemble_output(results) -> np.ndarray:
    # results[c]["out"]: [PAIRS, 128, NBLK, E]; s = 128*k + partition
    arr = np.stack([np.asarray(r["out"]) for r in results])
    arr = arr.reshape(B * H, 128, NBLK, E).astype(np.float32)
    arr = arr.transpose(0, 2, 1, 3).reshape(B, H, L, E).transpose(0, 2, 1, 3)
    return np.ascontiguousarray(arr)


def kernel(queries=None, keys=None, values=None, w_score=None, b_score=None, attn_mask=None, **_):
    global LAST_RESULTS
    from concourse.bass_utils import run_bass_kernel_spmd

    nc = _get_compiled()
    in_maps = prep_inputs(keys, values, w_score)
    res = run_bass_kernel_spmd(nc, in_maps, core_ids=list(range(NCORES)), trace=TRACE)
    LAST_RESULTS = res
    return assemble_output(res.results)
